# revision 22
# baseline (speedup 1.0000x reference)
"""Trainium2 Bass kernel for nn_ConsistencyConstraint (loss_fn).

Reference computation (B=4096, D=C*H*W=4096, NCLASS=10):
    ngrad_i = (g_i - min_i) / (max_i - min_i)          per-row min-max norm
    vn_i    = ngrad_i / max(||ngrad_i||, eps)
    sim     = vn @ vn.T
    xloss   = sum_{i<j, pred_i==pred_j} (1 - sim_ij) / B
    celoss  = mean cross-entropy(outputs, y)
    loss    = celoss + xloss

Key restructuring (mathematically identical; ~1e-4 rel err against the fp32
reference, which itself carries ~2e-5 fp32 noise):

1. Cosine similarity is invariant to the per-row positive scale 1/(max-min),
   so vn_i = z_i / ||z_i|| with z_i = g_i - min_i (the eps clamp is inactive:
   min-max normalized rows always have norm >= 1).
2. For same-class pairs: sum_{i<j in c} vn_i.vn_j = (||S_c||^2 - n_c) / 2
   where S_c = sum_{i in c} vn_i and sum_c n_c = B (each ||vn_i||^2 == 1), so
       xloss = (N_pairs - (sum_c ||S_c||^2 - B) / 2) / B,
       N_pairs = sum_c n_c (n_c - 1) / 2.
   This replaces the O(B^2 D) similarity matmul with an O(B D NCLASS)
   one-hot matmul.
Per-core dataflow (512 rows, 4 chunks of 128 partitions):
  - DVE:  row min reduce (two column halves to overlap the chunk DMA);
          z = g - min (fp16) on even chunks.
  - ACT:  ||z_i||^2 in ONE pass: Square activation with bias=-min_i
          (per-partition) and free-dim accumulate; z-pass on odd chunks
          (Identity with bias=-min); CE exp.
  - PE:   8 PSUM banks accumulate S = Wa^T @ Z over the 4 chunks, with
          Wa[i,c] = [argmax(outputs_i)==c] / ||z_i|| (fp16 stationary);
          PSUM is DMA'd straight to DRAM.
  - CE / argmax bookkeeping batched across chunks as single [128,4,10] ops
    using stride-0 broadcast access patterns.
Host gather: S = sum over cores, ||S_c||^2, bincount preds for N_pairs,
celoss rows = log(se) + (max_o - o_y). No device collectives.
"""

import numpy as np

import concourse.bass as bass
import concourse.mybir as mybir
import concourse.tile as tile
from concourse import bacc
from concourse.bass_utils import run_bass_kernel_spmd

N_CORES = 8
B = 4096
D = 4096  # C*H*W = 1*64*64
NCLASS = 10
ROWS_PER_CORE = B // N_CORES  # 512
P = 128  # SBUF partitions
KCH = ROWS_PER_CORE // P  # 4 row-chunks per core
NFREE = 512  # PSUM bank width (fp32)
NCH = D // NFREE  # 8 column-chunks
DH = D // 2  # DMA column-half

F32 = mybir.dt.float32
FP16 = mybir.dt.float16

# Results of the last device run (BassKernelResults) — exposed so an external
# harness can read exec_time_ns when tracing is enabled via BASS_TRACE=1.
LAST_RESULTS = None

_nc_cache = None


def _bc(ap, pattern):
    """Rebuild an AP with an explicit [step, count] pattern (for stride-0
    broadcasts along free dims)."""
    return bass.AP(tensor=ap.tensor, offset=ap.offset, ap=pattern)


def _build_bass():
    """One SPMD program, identical on all 8 cores; only the data differs."""
    nc = bacc.Bacc()

    g_in = nc.dram_tensor("g", [ROWS_PER_CORE, D], F32, kind="ExternalInput")
    o_in = nc.dram_tensor("o", [ROWS_PER_CORE, NCLASS], F32, kind="ExternalInput")
    # y as float (values 0..9), pre-laid-out [P, KCH] with [p, k] = y[k*128+p]
    y_in = nc.dram_tensor("yf", [P, KCH], F32, kind="ExternalInput")

    p_out = nc.dram_tensor("P", [NCLASS, D], F32, kind="ExternalOutput")
    se_out = nc.dram_tensor("se", [P, KCH], F32, kind="ExternalOutput")
    dm_out = nc.dram_tensor("dm", [P, KCH], F32, kind="ExternalOutput")
    pred_out = nc.dram_tensor("pred", [P, KCH], F32, kind="ExternalOutput")

    iota_const = nc.inline_tensor(
        np.tile(np.arange(NCLASS, dtype=np.float32), (P, 1)), name="iota10"
    )

    with tile.TileContext(nc) as tc:
        with (
            tc.tile_pool(name="gpool", bufs=4) as gpool,
            tc.tile_pool(name="zpool", bufs=4) as zpool,
            tc.tile_pool(name="jpool", bufs=2) as jpool,
            tc.tile_pool(name="small", bufs=4) as small,
            tc.tile_pool(name="singles", bufs=1) as singles,
            tc.tile_pool(name="outp", bufs=1) as outp,
            tc.tile_pool(name="psum", bufs=1, space="PSUM") as psum,
        ):
            # g chunk loads first — they own the DMA pipes from t=0.
            gts = []
            for k in range(KCH):
                gt = gpool.tile([P, D], F32, tag="gt", name=f"gt{k}")
                rows = slice(k * P, (k + 1) * P)
                nc.sync.dma_start(out=gt[:, :DH], in_=g_in[rows, :DH])
                nc.sync.dma_start(out=gt[:, DH:], in_=g_in[rows, DH:])
                gts.append(gt)

            iota_sb = singles.tile([P, NCLASS], F32)
            nc.sync.dma_start(out=iota_sb, in_=iota_const[:, :])
            yf_sb = singles.tile([P, KCH], F32)
            nc.sync.dma_start(out=yf_sb, in_=y_in[:, :])
            # o as [p, k, c] = outputs[k*128+p, c]
            o_all = singles.tile([P, KCH, NCLASS], F32)
            nc.sync.dma_start(
                out=o_all, in_=o_in.rearrange("(k p) c -> p k c", p=P)
            )

            se_sb = outp.tile([P, KCH], F32)
            dm_sb = outp.tile([P, KCH], F32)
            pred_sb = outp.tile([P, KCH], F32)
            p_sb = outp.tile([NCLASS, D], F32)

            acc = [
                psum.tile([NCLASS, NFREE], F32, tag=f"acc{n}", name=f"acc{n}")
                for n in range(NCH)
            ]

            # ---- batched argmax one-hot + CE bookkeeping (all 4 chunks) ----
            mo_all = small.tile([P, KCH], F32)
            nc.vector.tensor_reduce(
                mo_all, o_all, axis=mybir.AxisListType.X, op=mybir.AluOpType.max
            )
            mo_b = _bc(mo_all[:, :], [*mo_all[:, :].ap, [0, NCLASS]])
            eq_all = small.tile([P, KCH, NCLASS], FP16)
            nc.vector.tensor_tensor(
                eq_all, o_all, mo_b, op=mybir.AluOpType.is_equal
            )

            iota_b = _bc(
                iota_sb[:, :],
                [iota_sb[:, :].ap[0], [0, KCH], iota_sb[:, :].ap[1]],
            )
            # pred_i = sum_c c * onehot[i,c]  (ties have prob ~0 for randn)
            pp_all = small.tile([P, KCH, NCLASS], F32)
            nc.vector.tensor_tensor(pp_all, eq_all, iota_b, op=mybir.AluOpType.mult)
            nc.vector.tensor_reduce(
                pred_sb, pp_all, axis=mybir.AxisListType.X, op=mybir.AluOpType.add
            )

            # CE: se = sum_c exp(o - max_o); dm = max_o - o[y]
            emo = small.tile([P, KCH, NCLASS], F32)
            nc.vector.tensor_tensor(emo, o_all, mo_b, op=mybir.AluOpType.subtract)
            et = small.tile([P, KCH, NCLASS], F32)
            nc.scalar.activation(et, emo, mybir.ActivationFunctionType.Exp)
            nc.vector.tensor_reduce(
                se_sb, et, axis=mybir.AxisListType.X, op=mybir.AluOpType.add
            )
            yf_b = _bc(yf_sb[:, :], [*yf_sb[:, :].ap, [0, NCLASS]])
            ohy = small.tile([P, KCH, NCLASS], F32)
            nc.vector.tensor_tensor(ohy, iota_b, yf_b, op=mybir.AluOpType.is_equal)
            oyp = small.tile([P, KCH, NCLASS], F32)
            nc.vector.tensor_tensor(oyp, o_all, ohy, op=mybir.AluOpType.mult)
            oy_all = small.tile([P, KCH], F32)
            nc.vector.tensor_reduce(
                oy_all, oyp, axis=mybir.AxisListType.X, op=mybir.AluOpType.add
            )
            nc.vector.tensor_sub(dm_sb, mo_all, oy_all)

            # ---- main per-chunk pipeline over grad ----
            for k in range(KCH):
                gt = gts[k]

                # row min: elementwise min of the two column halves (half the
                # reduce traffic), then one [P, DH] reduce
                mh = jpool.tile([P, DH], F32, tag="mh")
                nc.vector.tensor_tensor(
                    mh, gt[:, :DH], gt[:, DH:], op=mybir.AluOpType.min
                )
                mn = small.tile([P, 1], F32, tag="mn")
                nc.vector.tensor_reduce(
                    mn, mh, axis=mybir.AxisListType.X, op=mybir.AluOpType.min
                )
                negm = small.tile([P, 1], F32, tag="negm")
                nc.vector.tensor_scalar_mul(negm, mn, -1.0)

                # z = g - min (fp16) on DVE (keeps ACT's activation-table
                # stable: Square/Sqrt only, no Identity swaps)
                zt = zpool.tile([P, D], FP16, tag="zt")
                nc.vector.tensor_scalar(
                    zt, gt, scalar1=mn, scalar2=None,
                    op0=mybir.AluOpType.subtract,
                )

                # ||z||^2 = sum((g - min)^2) in one ACT pass
                junk = jpool.tile([P, D], FP16, tag="junk")
                ssq = small.tile([P, 1], F32, tag="ssq")
                nc.scalar.activation(
                    junk,
                    gt,
                    mybir.ActivationFunctionType.Square,
                    bias=negm,
                    accum_out=ssq,
                )
                u = small.tile([P, 1], F32, tag="u")
                nc.scalar.activation(u, ssq, mybir.ActivationFunctionType.Sqrt)
                rs = small.tile([P, 1], F32, tag="rs")
                nc.vector.reciprocal(rs, u)

                # wa = onehot * (1/||z||), fp16 stationary operand
                wa = small.tile([P, NCLASS], FP16, tag="wa")
                nc.vector.tensor_scalar_mul(wa, eq_all[:, k, :], rs)

                for n in range(NCH):
                    nc.tensor.matmul(
                        acc[n][:, :],
                        wa,
                        zt[:, n * NFREE : (n + 1) * NFREE],
                        start=(k == 0),
                        stop=(k == KCH - 1),
                    )

            # ---- drain PSUM -> SBUF -> DRAM (copies split across engines) ----
            for n in range(NCH):
                dst = p_sb[:, n * NFREE : (n + 1) * NFREE]
                if n % 2 == 0:
                    nc.scalar.copy(dst, acc[n])
                else:
                    nc.vector.tensor_copy(dst, acc[n])
            nc.sync.dma_start(out=p_out[:, :], in_=p_sb)
            nc.sync.dma_start(out=se_out[:, :], in_=se_sb)
            nc.sync.dma_start(out=dm_out[:, :], in_=dm_sb)
            nc.sync.dma_start(out=pred_out[:, :], in_=pred_sb)

    nc.compile()
    return nc


def kernel(**inputs) -> np.ndarray:
    global LAST_RESULTS, _nc_cache

    outputs = np.ascontiguousarray(np.asarray(inputs["outputs"], dtype=np.float32))
    grad = np.asarray(inputs["grad"], dtype=np.float32).reshape(B, D)
    y = np.asarray(inputs["y"])

    if _nc_cache is None:
        _nc_cache = _build_bass()
    nc = _nc_cache

    yf = y.astype(np.float32)
    in_maps = []
    for c in range(N_CORES):
        sl = slice(c * ROWS_PER_CORE, (c + 1) * ROWS_PER_CORE)
        in_maps.append(
            {
                "g": np.ascontiguousarray(grad[sl]),
                "o": np.ascontiguousarray(outputs[sl]),
                # [p, k] = y[row k*128+p] to match the per-chunk partition layout
                "yf": np.ascontiguousarray(yf[sl].reshape(KCH, P).T),
            }
        )

    res = run_bass_kernel_spmd(nc, in_maps, core_ids=list(range(N_CORES)))
    LAST_RESULTS = res
    results = res.results

    # ---- host gather / unshard ----
    s_full = np.zeros((NCLASS, D), dtype=np.float64)
    ce_sum = 0.0
    preds = []
    for r in results:
        s_full += r["P"].astype(np.float64)
        ce_sum += float((np.log(r["se"].astype(np.float64)) + r["dm"]).sum())
        preds.append(r["pred"].astype(np.int64).reshape(-1))
    pred = np.concatenate(preds)
    counts = np.bincount(pred, minlength=max(NCLASS, int(pred.max()) + 1))

    n_pairs = float(
        (counts.astype(np.float64) * (counts.astype(np.float64) - 1) / 2).sum()
    )
    xsum = float((s_full * s_full).sum())
    xloss = (n_pairs - (xsum - B) / 2.0) / B
    celoss = ce_sum / B
    return np.float32(celoss + xloss)


# revision 23
# speedup vs baseline: 1.0459x; 1.0459x over previous
"""Trainium2 Bass kernel for nn_ConsistencyConstraint (loss_fn).

Reference computation (B=4096, D=C*H*W=4096, NCLASS=10):
    ngrad_i = (g_i - min_i) / (max_i - min_i)          per-row min-max norm
    vn_i    = ngrad_i / max(||ngrad_i||, eps)
    sim     = vn @ vn.T
    xloss   = sum_{i<j, pred_i==pred_j} (1 - sim_ij) / B
    celoss  = mean cross-entropy(outputs, y)
    loss    = celoss + xloss

Key restructuring (mathematically identical; ~1e-4 rel err against the fp32
reference, which itself carries ~2e-5 fp32 noise):

1. Cosine similarity is invariant to the per-row positive scale 1/(max-min),
   so vn_i = z_i / ||z_i|| with z_i = g_i - min_i (the eps clamp is inactive:
   min-max normalized rows always have norm >= 1).
2. For same-class pairs: sum_{i<j in c} vn_i.vn_j = (||S_c||^2 - n_c) / 2
   where S_c = sum_{i in c} vn_i and sum_c n_c = B (each ||vn_i||^2 == 1), so
       xloss = (N_pairs - (sum_c ||S_c||^2 - B) / 2) / B,
       N_pairs = sum_c n_c (n_c - 1) / 2.
   This replaces the O(B^2 D) similarity matmul with an O(B D NCLASS)
   one-hot matmul.
Per-core dataflow (512 rows, 4 chunks of 128 partitions):
  - DVE:  row min reduce (two column halves to overlap the chunk DMA);
          z = g - min (fp16) on even chunks.
  - ACT:  ||z_i||^2 in ONE pass: Square activation with bias=-min_i
          (per-partition) and free-dim accumulate; z-pass on odd chunks
          (Identity with bias=-min); CE exp.
  - PE:   8 PSUM banks accumulate S = Wa^T @ Z over the 4 chunks, with
          Wa[i,c] = [argmax(outputs_i)==c] / ||z_i|| (fp16 stationary);
          PSUM is DMA'd straight to DRAM.
  - CE / argmax bookkeeping batched across chunks as single [128,4,10] ops
    using stride-0 broadcast access patterns.
Host gather: S = sum over cores, ||S_c||^2, bincount preds for N_pairs,
celoss rows = log(se) + (max_o - o_y). No device collectives.
"""

import numpy as np

import concourse.bass as bass
import concourse.mybir as mybir
import concourse.tile as tile
from concourse import bacc
from concourse.bass_utils import run_bass_kernel_spmd

N_CORES = 8
B = 4096
D = 4096  # C*H*W = 1*64*64
NCLASS = 10
ROWS_PER_CORE = B // N_CORES  # 512
P = 128  # SBUF partitions
KCH = ROWS_PER_CORE // P  # 4 row-chunks per core
NFREE = 512  # PSUM bank width (fp32)
NCH = D // NFREE  # 8 column-chunks
DH = D // 2  # DMA column-half

F32 = mybir.dt.float32
FP16 = mybir.dt.float16

# Results of the last device run (BassKernelResults) — exposed so an external
# harness can read exec_time_ns when tracing is enabled via BASS_TRACE=1.
LAST_RESULTS = None

_nc_cache = None


def _bc(ap, pattern):
    """Rebuild an AP with an explicit [step, count] pattern (for stride-0
    broadcasts along free dims)."""
    return bass.AP(tensor=ap.tensor, offset=ap.offset, ap=pattern)


def _build_bass():
    """One SPMD program, identical on all 8 cores; only the data differs."""
    nc = bacc.Bacc()

    g_in = nc.dram_tensor("g", [ROWS_PER_CORE, D], F32, kind="ExternalInput")
    o_in = nc.dram_tensor("o", [ROWS_PER_CORE, NCLASS], F32, kind="ExternalInput")
    # y as float (values 0..9), pre-laid-out [P, KCH] with [p, k] = y[k*128+p]
    y_in = nc.dram_tensor("yf", [P, KCH], F32, kind="ExternalInput")

    p_out = nc.dram_tensor("P", [NCLASS, D], F32, kind="ExternalOutput")
    se_out = nc.dram_tensor("se", [P, KCH], F32, kind="ExternalOutput")
    dm_out = nc.dram_tensor("dm", [P, KCH], F32, kind="ExternalOutput")
    pred_out = nc.dram_tensor("pred", [P, KCH], F32, kind="ExternalOutput")

    iota_const = nc.inline_tensor(
        np.tile(np.arange(NCLASS, dtype=np.float32), (P, 1)), name="iota10"
    )

    with tile.TileContext(nc) as tc:
        with (
            tc.tile_pool(name="gpool", bufs=4) as gpool,
            tc.tile_pool(name="zpool", bufs=4) as zpool,
            tc.tile_pool(name="jpool", bufs=2) as jpool,
            tc.tile_pool(name="small", bufs=4) as small,
            tc.tile_pool(name="singles", bufs=1) as singles,
            tc.tile_pool(name="outp", bufs=1) as outp,
            tc.tile_pool(name="psum", bufs=1, space="PSUM") as psum,
        ):
            # g chunk loads first — they own the DMA pipes from t=0.
            gts = []
            for k in range(KCH):
                gt = gpool.tile([P, D], F32, tag="gt", name=f"gt{k}")
                rows = slice(k * P, (k + 1) * P)
                nc.sync.dma_start(out=gt[:, :DH], in_=g_in[rows, :DH])
                nc.sync.dma_start(out=gt[:, DH:], in_=g_in[rows, DH:])
                gts.append(gt)

            iota_sb = singles.tile([P, NCLASS], F32)
            nc.sync.dma_start(out=iota_sb, in_=iota_const[:, :])
            yf_sb = singles.tile([P, KCH], F32)
            nc.sync.dma_start(out=yf_sb, in_=y_in[:, :])
            # o as [p, k, c] = outputs[k*128+p, c]
            o_all = singles.tile([P, KCH, NCLASS], F32)
            nc.sync.dma_start(
                out=o_all, in_=o_in.rearrange("(k p) c -> p k c", p=P)
            )

            se_sb = outp.tile([P, KCH], F32)
            dm_sb = outp.tile([P, KCH], F32)
            pred_sb = outp.tile([P, KCH], F32)
            p_sb = outp.tile([NCLASS, D], F32)

            acc = [
                psum.tile([NCLASS, NFREE], F32, tag=f"acc{n}", name=f"acc{n}")
                for n in range(NCH)
            ]

            # ---- batched argmax one-hot + CE bookkeeping (all 4 chunks) ----
            mo_all = small.tile([P, KCH], F32)
            nc.vector.tensor_reduce(
                mo_all, o_all, axis=mybir.AxisListType.X, op=mybir.AluOpType.max
            )
            mo_b = _bc(mo_all[:, :], [*mo_all[:, :].ap, [0, NCLASS]])
            eq_all = small.tile([P, KCH, NCLASS], FP16)
            nc.vector.tensor_tensor(
                eq_all, o_all, mo_b, op=mybir.AluOpType.is_equal
            )

            iota_b = _bc(
                iota_sb[:, :],
                [iota_sb[:, :].ap[0], [0, KCH], iota_sb[:, :].ap[1]],
            )
            # pred_i = sum_c c * onehot[i,c]  (ties have prob ~0 for randn)
            pp_all = small.tile([P, KCH, NCLASS], F32)
            nc.vector.tensor_tensor(pp_all, eq_all, iota_b, op=mybir.AluOpType.mult)
            nc.vector.tensor_reduce(
                pred_sb, pp_all, axis=mybir.AxisListType.X, op=mybir.AluOpType.add
            )

            # CE: se = sum_c exp(o - max_o); dm = max_o - o[y]
            emo = small.tile([P, KCH, NCLASS], F32)
            nc.vector.tensor_tensor(emo, o_all, mo_b, op=mybir.AluOpType.subtract)
            et = small.tile([P, KCH, NCLASS], F32)
            nc.scalar.activation(et, emo, mybir.ActivationFunctionType.Exp)
            nc.vector.tensor_reduce(
                se_sb, et, axis=mybir.AxisListType.X, op=mybir.AluOpType.add
            )
            yf_b = _bc(yf_sb[:, :], [*yf_sb[:, :].ap, [0, NCLASS]])
            ohy = small.tile([P, KCH, NCLASS], F32)
            nc.vector.tensor_tensor(ohy, iota_b, yf_b, op=mybir.AluOpType.is_equal)
            oyp = small.tile([P, KCH, NCLASS], F32)
            nc.vector.tensor_tensor(oyp, o_all, ohy, op=mybir.AluOpType.mult)
            oy_all = small.tile([P, KCH], F32)
            nc.vector.tensor_reduce(
                oy_all, oyp, axis=mybir.AxisListType.X, op=mybir.AluOpType.add
            )
            nc.vector.tensor_sub(dm_sb, mo_all, oy_all)

            # ---- main per-chunk pipeline over grad ----
            for k in range(KCH):
                gt = gts[k]

                # per-half min (each waits only its half's DMA), then combine
                mnh = small.tile([P, 2], F32, tag="mnh")
                nc.vector.tensor_reduce(
                    mnh[:, 0:1], gt[:, :DH], axis=mybir.AxisListType.X,
                    op=mybir.AluOpType.min,
                )
                nc.vector.tensor_reduce(
                    mnh[:, 1:2], gt[:, DH:], axis=mybir.AxisListType.X,
                    op=mybir.AluOpType.min,
                )
                mn = small.tile([P, 1], F32, tag="mn")
                nc.vector.tensor_tensor(
                    mn, mnh[:, 0:1], mnh[:, 1:2], op=mybir.AluOpType.min
                )
                negm = small.tile([P, 1], F32, tag="negm")
                nc.vector.tensor_scalar_mul(negm, mn, -1.0)

                # z = g - min (fp16) on DVE (keeps ACT's activation-table
                # stable: Square/Sqrt only, no Identity swaps)
                zt = zpool.tile([P, D], FP16, tag="zt")
                nc.vector.tensor_scalar(
                    zt, gt, scalar1=mn, scalar2=None,
                    op0=mybir.AluOpType.subtract,
                )

                # ||z||^2 = sum((g - min)^2) in one ACT pass
                junk = jpool.tile([P, D], FP16, tag="junk")
                ssq = small.tile([P, 1], F32, tag="ssq")
                nc.scalar.activation(
                    junk,
                    gt,
                    mybir.ActivationFunctionType.Square,
                    bias=negm,
                    accum_out=ssq,
                )
                u = small.tile([P, 1], F32, tag="u")
                nc.scalar.activation(u, ssq, mybir.ActivationFunctionType.Sqrt)
                rs = small.tile([P, 1], F32, tag="rs")
                nc.vector.reciprocal(rs, u)

                # wa = onehot * (1/||z||), fp16 stationary operand
                wa = small.tile([P, NCLASS], FP16, tag="wa")
                nc.vector.tensor_scalar_mul(wa, eq_all[:, k, :], rs)

                for n in range(NCH):
                    nc.tensor.matmul(
                        acc[n][:, :],
                        wa,
                        zt[:, n * NFREE : (n + 1) * NFREE],
                        start=(k == 0),
                        stop=(k == KCH - 1),
                    )

            # ---- drain PSUM -> SBUF -> DRAM (copies split across engines) ----
            for n in range(NCH):
                dst = p_sb[:, n * NFREE : (n + 1) * NFREE]
                if n % 2 == 0:
                    nc.scalar.copy(dst, acc[n])
                else:
                    nc.vector.tensor_copy(dst, acc[n])
            nc.sync.dma_start(out=p_out[:, :], in_=p_sb)
            nc.sync.dma_start(out=se_out[:, :], in_=se_sb)
            nc.sync.dma_start(out=dm_out[:, :], in_=dm_sb)
            nc.sync.dma_start(out=pred_out[:, :], in_=pred_sb)

    nc.compile()
    return nc


def kernel(**inputs) -> np.ndarray:
    global LAST_RESULTS, _nc_cache

    outputs = np.ascontiguousarray(np.asarray(inputs["outputs"], dtype=np.float32))
    grad = np.asarray(inputs["grad"], dtype=np.float32).reshape(B, D)
    y = np.asarray(inputs["y"])

    if _nc_cache is None:
        _nc_cache = _build_bass()
    nc = _nc_cache

    yf = y.astype(np.float32)
    in_maps = []
    for c in range(N_CORES):
        sl = slice(c * ROWS_PER_CORE, (c + 1) * ROWS_PER_CORE)
        in_maps.append(
            {
                "g": np.ascontiguousarray(grad[sl]),
                "o": np.ascontiguousarray(outputs[sl]),
                # [p, k] = y[row k*128+p] to match the per-chunk partition layout
                "yf": np.ascontiguousarray(yf[sl].reshape(KCH, P).T),
            }
        )

    res = run_bass_kernel_spmd(nc, in_maps, core_ids=list(range(N_CORES)))
    LAST_RESULTS = res
    results = res.results

    # ---- host gather / unshard ----
    s_full = np.zeros((NCLASS, D), dtype=np.float64)
    ce_sum = 0.0
    preds = []
    for r in results:
        s_full += r["P"].astype(np.float64)
        ce_sum += float((np.log(r["se"].astype(np.float64)) + r["dm"]).sum())
        preds.append(r["pred"].astype(np.int64).reshape(-1))
    pred = np.concatenate(preds)
    counts = np.bincount(pred, minlength=max(NCLASS, int(pred.max()) + 1))

    n_pairs = float(
        (counts.astype(np.float64) * (counts.astype(np.float64) - 1) / 2).sum()
    )
    xsum = float((s_full * s_full).sum())
    xloss = (n_pairs - (xsum - B) / 2.0) / B
    celoss = ce_sum / B
    return np.float32(celoss + xloss)
